# revision 2
# baseline (speedup 1.0000x reference)
"""Context-aware attention pooling kernel for Trainium2 (8 NeuronCores).

Reference computation (per batch b):
    e      = tanh(seq @ W1[:256] + ctx @ W1[256:])      # [T, 64]
    logits = e @ W2                                      # [T, 1]
    a      = softmax(logits over T)
    out    = sum_t a[t] * seq[t]                         # [256]

Shapes: B=64, T=4096, D1=256, D2=128, UNITS=64.
Sharding: data-parallel over batch, 8 batches per core; W1/W2 replicated.

Per-core program (all t-tiles are 128 rows):
  - seq[b] loaded in natural layout [t, d] as bf16 (f32->bf16 cast inside the
    SWDGE DMA); t is loaded permuted as t = 512m + 4p + s so each HBM
    descriptor covers 4 consecutive t rows (4 KiB contiguous). The whole
    pipeline is invariant to any fixed t-permutation as long as every stage
    indexes the same tile layout.
  - pair-transpose: adjacent-d bf16 pairs are reinterpreted as one f32 and
    PE-transposed as f32 blocks (one [128, 128] transpose per t-tile instead
    of two); the e-matmul reads the pair layout with stride-2 bf16 APs
    against even/odd-row-interleaved W1 copies
  - e-matmul contracts d on PE: eT2[u-half, t] (tanh + ctx-bias on ScalarE),
    with even/odd 512-t chunks col-packed into both halves of the PE array
  - logits: one [K=128, M=128, N=2] matmul per 128-col block of eT2 against a
    two-column W2 ([w2;0] | [0;w2]) -- each bf16 128-col LDWEIGHTS (FWL) covers
    256 t-values, replacing 4 single-column matmuls with 128-col weight loads
  - softmax without max-subtraction (|logit| <= ||W2||_1, safe in f32); one
    fused Exp + per-partition accumulate over the whole [128, 32] logit block;
    total Z via a ones-matmul; the single 1/Z scale is applied at the end
  - pooling on PE with 4-way column-group concurrency: tile n accumulates in
    col-group n%4 (PSUM partition 32*(n%4)), so up to 4 pool matmuls execute
    simultaneously in disjoint quadrant columns; the 4 partial rows are summed
    by a ones-matmul after evacuation to a zeroed SBUF tile
  - PE warm-up (HAM clock ramp) uses a memset tile so it starts immediately,
    independent of any DMA
"""

import numpy as np

import concourse.bacc as bacc
import concourse.mybir as mybir
from concourse.tile import TileContext

F32 = mybir.dt.float32
BF16 = mybir.dt.bfloat16

N_CORES = 8
B_CORE = 8          # batches per core
T = 4096
D1 = 256
D2 = 128
U = 64
NT = T // 128       # 32 t-tiles per batch


def build_program():
    nc = bacc.Bacc("TRN2", target_bir_lowering=False, debug=False)

    seq = nc.declare_dram_parameter("seq", [B_CORE, T, D1], F32, isOutput=False)
    ctx = nc.declare_dram_parameter("ctx", [B_CORE, D2], F32, isOutput=False)
    w1 = nc.declare_dram_parameter("w1", [D1 + D2, U], F32, isOutput=False)
    w2 = nc.declare_dram_parameter("w2", [U, 1], F32, isOutput=False)
    # identity fed as data (np.eye) so the GpSimd queue never stalls the seq
    # descriptor stream on an affine_select
    ident_in = nc.declare_dram_parameter("ident_in", [128, 128], F32, isOutput=False)
    outp = nc.declare_dram_parameter("outp", [1, B_CORE * D1], F32, isOutput=True)

    with TileContext(nc) as tc:
        with (
            tc.tile_pool(name="singles", bufs=1) as singles,
            tc.tile_pool(name="nat_pool", bufs=4) as nat_pool,
            tc.tile_pool(name="seqt_pool", bufs=2) as seqt_pool,
            tc.tile_pool(name="et_pool", bufs=2) as et_pool,
            tc.tile_pool(name="small_pool", bufs=2) as small_pool,
            tc.tile_pool(name="ps", bufs=1, space="PSUM") as ps,
        ):
            # HAM warm-up on a memset tile: starts immediately (no DMA dep) and
            # keeps the PE clock ramping while the first seq chunks stream in
            warm_sb = singles.tile([128, 128], F32)
            nc.vector.memset(warm_sb, 0.25)
            warm_ps = ps.tile([128, 128], F32, tag="z", bufs=1)
            for _ in range(32):
                nc.tensor.matmul(warm_ps, lhsT=warm_sb, rhs=warm_sb, start=True, stop=True)

            # identity via HWDGE (independent of the gpsimd queue)
            ident = singles.tile([128, 128], F32)
            nc.sync.dma_start(out=ident, in_=ident_in[:, :])
            ident8 = ident[0:8, 0:8]

            # W1[0:256] interleaved as [q, (s u)]: cols 0:64 = even rows
            # (d = 2q), cols 64:128 = odd rows (d = 2q+1); SWDGE handles the
            # 3D access pattern and the f32->bf16 cast. This is the only
            # setup work on the gpsimd queue ahead of the seq loads.
            w1eo = singles.tile([128, 2 * U], BF16)
            nc.gpsimd.dma_start(
                out=w1eo.rearrange("q (s u) -> q s u", s=2),
                in_=w1[0:256].rearrange("(q s) u -> q s u", s=2),
            )

            # ---- seq loads (natural layout, f32 -> bf16 cast in the DMA);
            # each batch is 4 chunks so consumers start on partial data.
            # t = 512m + 4p + s: each (p, m) descriptor covers 4 consecutive
            # t rows = 4 KiB contiguous HBM. Tile n = 4m + s holds
            # t = 512*(n//4) + 4p + (n%4).
            nat_tiles = [None] * B_CORE

            def load_nat(b):
                nat = nat_pool.tile(
                    [128, NT * D1], BF16, tag="nat", name=f"nat{b}"
                )
                seq_b = seq[b].rearrange("(m p s) d -> p m (s d)", p=128, s=4)
                nat_3d = nat.rearrange("p (m sd) -> p m sd", sd=4 * D1)
                for q in range(4):
                    nsl = slice(2 * q, 2 * (q + 1))
                    nc.gpsimd.dma_start(out=nat_3d[:, nsl], in_=seq_b[:, nsl])
                nat_tiles[b] = nat

            load_nat(0)
            load_nat(1)
            load_nat(2)

            w1c = singles.tile([128, U], F32)
            nc.sync.dma_start(out=w1c, in_=w1[256:384, :])

            # two-column W2 for the block-logits matmul: col 0 = [w2; 0],
            # col 1 = [0; w2] (partition halves pick even/odd t-chunks)
            w2c2f = singles.tile([128, 2], F32)
            nc.vector.memset(w2c2f, 0.0)
            nc.sync.dma_start(out=w2c2f[0:U, 0:1], in_=w2[:, :])
            nc.sync.dma_start(out=w2c2f[U:128, 1:2], in_=w2[:, :])
            w2c2 = singles.tile([128, 2], BF16)
            nc.vector.tensor_copy(w2c2, w2c2f)

            ctx_nat = singles.tile([B_CORE, D2], F32)
            nc.sync.dma_start(out=ctx_nat, in_=ctx[:, :])
            ctxT_ps = ps.tile([D2, B_CORE], F32, tag="lg", bufs=1)
            nc.tensor.transpose(ctxT_ps, ctx_nat, ident8)
            ctxT = singles.tile([D2, B_CORE], F32)
            nc.vector.tensor_copy(ctxT, ctxT_ps)

            # all 8 context projections at once, duplicated on both partition
            # halves (tanh bias for even/odd chunks): cb_all[64h + u, b]
            cb_ps = ps.tile([128, B_CORE], F32, tag="pool", bufs=1)
            nc.tensor.matmul(cb_ps[0:U], lhsT=w1c, rhs=ctxT, start=True, stop=True)
            nc.tensor.matmul(
                cb_ps[U:128],
                lhsT=w1c,
                rhs=ctxT,
                start=True,
                stop=True,
                tile_position=(0, U),
            )
            cb_all = singles.tile([128, B_CORE], F32)
            nc.scalar.copy(cb_all, cb_ps)

            ones_col = singles.tile([128, 1], F32)
            nc.vector.memset(ones_col, 1.0)

            final_sb = singles.tile([1, B_CORE * D1], F32)

            # ---- per-batch pipeline ----
            for b in range(B_CORE):
                nat = nat_tiles[b]
                if b + 3 < B_CORE:
                    load_nat(b + 3)

                # Pair-transpose trick: reinterpret the bf16 pair
                # (seq[t, 2q], seq[t, 2q+1]) as one f32 and PE-transpose f32
                # blocks -- one [128, 128] transpose per t-tile instead of two.
                # seqTp[q, 2t + s] (bf16 view) = seq[t, 2q + s].
                nat_f32 = nat.bitcast(F32)
                seqTp = seqt_pool.tile([128, T], F32, tag="seqTp", name=f"sTp{b}")
                for k in range(NT // 4):
                    pst = ps.tile([128, 512], F32, tag="tp", bufs=2)
                    for i in range(4):
                        n = 4 * k + i
                        nc.tensor.transpose(
                            pst[:, 128 * i : 128 * (i + 1)],
                            nat_f32[:, 128 * n : 128 * (n + 1)],
                            ident,
                        )
                    nc.vector.tensor_copy(seqTp[:, 512 * k : 512 * (k + 1)], pst)
                # [128, s, t] bf16 view: s=0 -> even d rows, s=1 -> odd
                stp = seqTp.bitcast(BF16).rearrange("p (t s) -> p s t", s=2)

                # e = tanh(z + cb) as eT2 [128, 2048] bf16: even 512-chunks of
                # t on partitions 0..63, odd chunks on partitions 64..127 (so
                # logits matmuls can row-pack into both halves of the PE array)
                eT2 = et_pool.tile([128, T // 2], BF16, tag="eT2", name=f"eT2_{b}")
                for c in range(T // 512):
                    par = c % 2
                    rsl = slice(U * par, U * par + U)
                    e_ps = ps.tile([128, 512], F32, tag="e", bufs=2)
                    sl = slice(512 * c, 512 * (c + 1))
                    tp = (0, U * par)
                    nc.tensor.matmul(
                        e_ps[rsl],
                        lhsT=w1eo[:, 0:U],
                        rhs=stp[:, 0, sl],
                        start=True,
                        stop=False,
                        tile_position=tp,
                    )
                    nc.tensor.matmul(
                        e_ps[rsl],
                        lhsT=w1eo[:, U : 2 * U],
                        rhs=stp[:, 1, sl],
                        start=False,
                        stop=True,
                        tile_position=tp,
                    )
                    nc.scalar.activation(
                        eT2[rsl, 512 * (c // 2) : 512 * (c // 2) + 512],
                        e_ps[rsl],
                        mybir.ActivationFunctionType.Tanh,
                        bias=cb_all[rsl, b : b + 1],
                    )

                # block logits: eT2 cols [128j : 128j+128] hold even chunk
                # (rows 0:64) and odd chunk (rows 64:128) t-values; one matmul
                # against the two-column w2 yields lg2[:, 2j] = even-chunk
                # logits, lg2[:, 2j+1] = odd-chunk logits. Column mapping
                # matches the old lgA/lgB: tile n -> jj = 4*((n//4)//2) + n%4,
                # parity (n//4)%2, p2 column = 2*jj + parity.
                lg2 = ps.tile([128, 2 * (NT // 2)], F32, tag="lg", bufs=1)
                for j in range(16):
                    nc.tensor.matmul(
                        lg2[:, 2 * j : 2 * j + 2],
                        lhsT=eT2[:, 128 * j : 128 * (j + 1)],
                        rhs=w2c2,
                        start=True,
                        stop=True,
                    )

                # p = exp(logits), one fused op with per-partition accumulate
                p2 = small_pool.tile([128, 2 * (NT // 2)], BF16, tag="p2")
                psums = small_pool.tile([128, 1], F32, tag="psums")
                nc.scalar.activation(
                    p2, lg2, mybir.ActivationFunctionType.Exp, accum_out=psums
                )

                # Z = sum over partitions of psums
                z_ps = ps.tile([1, 1], F32, tag="z", bufs=1)
                nc.tensor.matmul(z_ps, lhsT=psums, rhs=ones_col, start=True, stop=True)
                invz = small_pool.tile([1, 1], F32, tag="invz")
                nc.vector.reciprocal(invz, z_ps)

                # pooling: out[d] = sum_t p[t] * seq[t, d], accumulated on PE
                # in 4 independent column-groups (4-way concurrent matmuls);
                # group g = n%4 accumulates at PSUM partition 32*g
                pool_ps = ps.tile([128, D1], F32, tag="pool", bufs=1)
                for n in range(NT):
                    c = n // 4
                    col = 2 * (4 * (c // 2) + n % 4) + (c % 2)
                    g = n % 4
                    nc.tensor.matmul(
                        pool_ps[32 * g : 32 * g + 1, :],
                        lhsT=p2[:, col : col + 1],
                        rhs=nat[:, 256 * n : 256 * (n + 1)],
                        start=(n < 4),
                        stop=(n >= NT - 4),
                        tile_position=(0, 32 * g),
                    )

                # combine the 4 partial rows: evacuate to a zeroed SBUF tile,
                # then a ones-matmul sums across partitions
                poolsb = small_pool.tile([128, D1], F32, tag="poolsb")
                nc.vector.memset(poolsb, 0.0)
                for g in range(4):
                    nc.scalar.copy(
                        poolsb[32 * g : 32 * g + 1, :],
                        pool_ps[32 * g : 32 * g + 1, :],
                    )
                final_ps = ps.tile([1, D1], F32, tag="fin", bufs=1)
                nc.tensor.matmul(
                    final_ps, lhsT=ones_col, rhs=poolsb, start=True, stop=True
                )

                # normalize by 1/Z while evacuating to SBUF, store per batch
                nc.scalar.activation(
                    final_sb[0:1, D1 * b : D1 * (b + 1)],
                    final_ps,
                    mybir.ActivationFunctionType.Copy,
                    scale=invz,
                )
                nc.sync.dma_start(
                    out=outp[0:1, D1 * b : D1 * (b + 1)],
                    in_=final_sb[0:1, D1 * b : D1 * (b + 1)],
                )

    nc.compile()
    return nc


_NC_CACHE = []


def _get_program():
    if not _NC_CACHE:
        _NC_CACHE.append(build_program())
    return _NC_CACHE[0]


def make_in_maps(sequence, context, W1, W2):
    ident = np.eye(128, dtype=np.float32)
    in_maps = []
    for c in range(N_CORES):
        sl = slice(B_CORE * c, B_CORE * (c + 1))
        in_maps.append(
            {
                "seq": np.ascontiguousarray(sequence[sl], dtype=np.float32),
                "ctx": np.ascontiguousarray(context[sl], dtype=np.float32),
                "w1": np.ascontiguousarray(W1, dtype=np.float32),
                "w2": np.ascontiguousarray(W2, dtype=np.float32),
                "ident_in": ident,
            }
        )
    return in_maps


def kernel(sequence, context, W1, W2):
    """Full-input entry point: shards batch across 8 cores, returns [64, 256] f32."""
    from concourse.bass_utils import run_bass_kernel_spmd

    nc = _get_program()
    in_maps = make_in_maps(sequence, context, W1, W2)
    res = run_bass_kernel_spmd(nc, in_maps, list(range(N_CORES)))
    out = np.concatenate(
        [res.results[c]["outp"].reshape(B_CORE, D1) for c in range(N_CORES)], axis=0
    )
    return out.astype(np.float32)
